# revision 24
# baseline (speedup 1.0000x reference)
"""GCN 2-layer forward on 8 Trainium2 NeuronCores (Bass/Tile).

Strategy (1D dst sharding, segment substreams + one-hot P aggregation):
  - norm dinv[src]*dinv[dst] is separable: dinv[src] folded into x rows on
    host; dinv[dst] applied per dst-block after aggregation.
  - Edges per core sorted by (dst block, src segment, dst). Gathers use the
    GPSIMD dma_gather (int16 idx) against <=32512-row table segments; each
    128-edge chunk is aggregated into its block's PSUM tile via a one-hot
    P matrix (tensor_scalar is_equal) matmul.
  - L1 per-block epilogue: t = dinv*(agg@W1); out1 = relu(t+b1);
    table2' = (dinv*out1)@W2 rows (fp16, padded to 128 cols = 256B rows).
  - AllGather table2' shards across cores (on-device collective).
  - L2: same aggregation over table2' rows; out = dinv*agg2 (+b2).
"""
import sys

sys.path.insert(0, "/opt/trn_rl_repo")

import numpy as np

import concourse.bass as bass
import concourse.bacc as bacc
import concourse.tile as tile
import concourse.mybir as mybir
from concourse.library_config import mlp as mlp_lib

P = 128
SEGSZ = 32512


class CFG:
    def __init__(self, N=100000, NF=128, H=64, C=40, NCORES=8, K=8,
                 segsz=SEGSZ):
        assert N % NCORES == 0
        self.N, self.NF, self.H, self.C, self.NCORES = N, NF, H, C, NCORES
        self.K = K
        self.SEGSZ = segsz
        self.SH = N // NCORES                  # real nodes per shard
        self.NB = (self.SH + P - 1) // P       # dst blocks per shard
        self.SHP = self.NB * P                 # padded shard rows
        self.TROW = NCORES * self.SHP          # table2p rows


def _build_layer_struct(cfg, rows_of_src, dst_l, core_of, nseg):
    """Chunk structure for one layer.

    rows_of_src: per-edge table row (int64), dst_l: per-edge local dst,
    core_of: per-edge owning core.  Returns (meta_struct, idx_dev, dl_dev)
    where idx_dev [NC, NGI_tot, P, K*8] i16, dl_dev [NC, NGI_tot, P, K] f16.
    """
    NC, NB, K = cfg.NCORES, cfg.NB, cfg.K
    seg = rows_of_src // cfg.SEGSZ
    rel = (rows_of_src - seg * cfg.SEGSZ).astype(np.int64)
    blk = dst_l >> 7

    # count per (core, block, seg)
    cnt = np.zeros((NC, NB, nseg), np.int64)
    np.add.at(cnt, (core_of, blk, seg), 1)
    R = np.maximum(1 if NB == 1 else 0, np.ceil(cnt.max(axis=0) / P).astype(np.int64))
    # filler chunk for final block's pad partitions (init PSUM pad rows)
    R[NB - 1, nseg - 1] += 1

    css = np.zeros((NB, nseg), np.int64)     # chunk start within substream
    L_s = np.zeros(nseg, np.int64)
    for s in range(nseg):
        css[:, s] = np.concatenate([[0], np.cumsum(R[:, s])[:-1]])
        L_s[s] = R[:, s].sum()
    NGI_s = [int(-(-L_s[s] // K)) for s in range(nseg)]
    gioff = np.concatenate([[0], np.cumsum(NGI_s)]).astype(np.int64)
    NGI_tot = int(gioff[-1])
    # pad substream tails to gather granularity -> extra pad chunks on last blk
    Rp = R.copy()
    for s in range(nseg):
        Rp[NB - 1, s] += NGI_s[s] * K - L_s[s]

    idx_lin = np.zeros((NC, nseg), object)
    dl_lin = np.zeros((NC, nseg), object)
    for c in range(NC):
        for s in range(nseg):
            n = NGI_s[s] * K * P
            idx_lin[c, s] = np.zeros(n, np.int64)
            dl_lin[c, s] = np.full(n, 999.0, np.float32)

    # fill edges: position within (core, block, seg) group
    order = np.lexsort((dst_l, seg, blk, core_of))
    co, bo, so = core_of[order], blk[order], seg[order]
    ro, do = rel[order], (dst_l[order] & 127)
    gid = (co * NB + bo) * nseg + so
    gstart = np.zeros(NC * NB * nseg + 1, np.int64)
    np.add.at(gstart, gid + 1, 1)
    gstart = np.cumsum(gstart)
    pos = np.arange(len(order)) - gstart[gid]
    ci = css[bo, so] + (pos >> 7)          # chunk within substream
    pp = pos & 127
    flat = ci * P + pp
    for c in range(NC):
        for s in range(nseg):
            m = (co == c) & (so == s)
            idx_lin[c, s][flat[m]] = ro[m]
            dl_lin[c, s][flat[m]] = do[m]

    # filler chunk: last block, last seg, chunk index css[NB-1, -1]+R_orig-1
    fill_ci = css[NB - 1, nseg - 1] + R[NB - 1, nseg - 1] - 1
    padlo = cfg.SH - (NB - 1) * P            # first pad partition in last blk
    for c in range(NC):
        a = dl_lin[c, nseg - 1]
        for p_ in range(padlo, P):
            a[fill_ci * P + p_] = float(p_)

    # device layout
    idx_dev = np.zeros((NC, NGI_tot, P, K * 8), np.int16)
    dl_dev = np.zeros((NC, NGI_tot, P, K), np.float32)
    for c in range(NC):
        for s in range(nseg):
            il = idx_lin[c, s].reshape(NGI_s[s], K * P)
            # wrapped: idx i at [i%16, i//16]; replicate over 8 groups
            wr = il.reshape(NGI_s[s], K * 8, 16).transpose(0, 2, 1)
            idx_dev[c, gioff[s]:gioff[s + 1], :, :] = np.tile(wr, (1, 8, 1)
                                                              ).astype(np.int16)
            # dl: [gi, p, j] = dstloc of chunk gi*K+j partition p
            dl = dl_lin[c, s].reshape(NGI_s[s], K, P).transpose(0, 2, 1)
            dl_dev[c, gioff[s]:gioff[s + 1], :, :] = dl.astype(np.float32)

    struct = dict(R=Rp, css=css, NGI_s=NGI_s, gioff=gioff, NGI_tot=NGI_tot,
                  nseg=nseg)
    return struct, idx_dev, dl_dev


def preprocess(cfg, x, edge_index, W1, b1, W2, b2):
    N, SH, NB, SHP = cfg.N, cfg.SH, cfg.NB, cfg.SHP
    NC = cfg.NCORES
    src = np.asarray(edge_index[0]).astype(np.int64)
    dst = np.asarray(edge_index[1]).astype(np.int64)
    loops = np.arange(N, dtype=np.int64)
    src_a = np.concatenate([src, loops])
    dst_a = np.concatenate([dst, loops])

    deg = np.bincount(dst_a, minlength=N).astype(np.float32)
    dinv = 1.0 / np.sqrt(deg)

    x16 = (np.asarray(x).astype(np.float32) * dinv[:, None]).astype(np.float16)

    core_of = dst_a // SH
    dst_l = dst_a - core_of * SH

    nseg1 = -(-N // cfg.SEGSZ)
    st1, idx1, dl1 = _build_layer_struct(cfg, src_a, dst_l, core_of, nseg1)

    g2row = SHP * (src_a // SH) + (src_a - (src_a // SH) * SH)
    nseg2 = -(-cfg.TROW // cfg.SEGSZ)
    st2, idx2, dl2 = _build_layer_struct(cfg, g2row, dst_l, core_of, nseg2)

    dinv_cols = np.zeros((NC, P, NB), np.float32)
    for c in range(NC):
        dp = np.zeros(SHP, np.float32)
        dp[:SH] = dinv[c * SH:(c + 1) * SH]
        dinv_cols[c] = dp.reshape(NB, P).T

    iota = np.broadcast_to(np.arange(P, dtype=np.float16), (P, P)).copy()
    ident = np.eye(P, dtype=np.float16)

    w1_16 = np.asarray(W1).astype(np.float16)
    w2_16 = np.asarray(W2).astype(np.float16)
    b1_bc = np.broadcast_to(np.asarray(b1).astype(np.float32), (P, cfg.H)).copy()
    b2_bc = np.broadcast_to(np.asarray(b2).astype(np.float32), (P, cfg.C)).copy()

    meta = dict(st1=st1, st2=st2,
                has_b1=bool(np.any(np.asarray(b1))),
                has_b2=bool(np.any(np.asarray(b2))))
    ins = dict(x16=x16, idx1=idx1, dl1=dl1, idx2=idx2, dl2=dl2,
               dinv_cols=dinv_cols, w1=w1_16, w2=w2_16, b1=b1_bc, b2=b2_bc,
               iota=iota, ident=ident)
    return meta, ins


def build_kernel(cfg, meta, nc, repeat=1, mock_cc=False, skip=()):
    f16, f32, i16 = mybir.dt.float16, mybir.dt.float32, mybir.dt.int16
    NF, H, C, NB, K = cfg.NF, cfg.H, cfg.C, cfg.NB, cfg.K
    st1, st2 = meta["st1"], meta["st2"]

    x16 = nc.dram_tensor("x16", [cfg.N, NF], f16, kind="ExternalInput").ap()
    idx1 = nc.dram_tensor("idx1", [st1["NGI_tot"], P, K * 8], i16,
                          kind="ExternalInput").ap()
    dl1 = nc.dram_tensor("dl1", [st1["NGI_tot"], P, K], f32,
                         kind="ExternalInput").ap()
    idx2 = nc.dram_tensor("idx2", [st2["NGI_tot"], P, K * 8], i16,
                          kind="ExternalInput").ap()
    dl2 = nc.dram_tensor("dl2", [st2["NGI_tot"], P, K], f32,
                         kind="ExternalInput").ap()
    dinvc = nc.dram_tensor("dinv_cols", [P, NB], f32, kind="ExternalInput").ap()
    w1 = nc.dram_tensor("w1", [NF, H], f16, kind="ExternalInput").ap()
    w2 = nc.dram_tensor("w2", [H, C], f16, kind="ExternalInput").ap()
    b1 = nc.dram_tensor("b1", [P, H], f32, kind="ExternalInput").ap()
    b2 = nc.dram_tensor("b2", [P, C], f32, kind="ExternalInput").ap()
    iota_in = nc.dram_tensor("iota", [P, P], f16, kind="ExternalInput").ap()
    ident_in = nc.dram_tensor("ident", [P, P], f16, kind="ExternalInput").ap()
    out = nc.dram_tensor("out", [cfg.SHP, C], f32, kind="ExternalOutput").ap()

    with tile.TileContext(nc) as tc:
        with (
            tc.tile_pool(name="const", bufs=1) as const,
            tc.tile_pool(name="dram", bufs=1, space="DRAM") as dram,
            tc.tile_pool(name="gp", bufs=3) as gp,
            tc.tile_pool(name="pp", bufs=6) as ppool,
            tc.tile_pool(name="epi", bufs=3) as epi,
            tc.tile_pool(name="psA", bufs=3, space="PSUM") as psA,
            tc.tile_pool(name="psB", bufs=2, space="PSUM") as psB,
        ):
            nc.gpsimd.load_library(mlp_lib)

            shard = dram.tile([cfg.SHP, P], f16)
            table2p = dram.tile(
                [cfg.TROW, P], f16,
                addr_space="Shared" if (cfg.NCORES > 4 and not mock_cc)
                else "Local")

            iota_t = const.tile([P, P], f16)
            nc.sync.dma_start(out=iota_t[:], in_=iota_in[:, :])
            ident = const.tile([P, P], f16)
            nc.sync.dma_start(out=ident[:], in_=ident_in[:, :])
            w1_t = const.tile([NF, H], f16)
            nc.sync.dma_start(out=w1_t[:], in_=w1[:, :])
            w2_t = const.tile([H, C], f16)
            nc.sync.dma_start(out=w2_t[:], in_=w2[:, :])
            dinv_t = const.tile([P, NB], f32)
            nc.sync.dma_start(out=dinv_t[:], in_=dinvc[:, :])
            b1_t = const.tile([P, H], f32)
            if meta["has_b1"]:
                nc.sync.dma_start(out=b1_t[:], in_=b1[:, :])
            b2_t = const.tile([P, C], f32)
            if meta["has_b2"]:
                nc.sync.dma_start(out=b2_t[:], in_=b2[:, :])

            def layer(st, idx_ap, dl_ap, src_ap, src_rows, Freal, drain):
                nseg = st["nseg"]
                R, css, gioff = st["R"], st["css"], st["gioff"]
                cur_gi = [-1] * nseg
                G = [None] * nseg
                DL = [None] * nseg
                for b in range(NB):
                    n_b = int(R[b].sum())
                    agg = psA.tile([P, NF], f32, space="PSUM", tag="agg",
                                   name="aggt")
                    t = 0
                    for s in range(nseg):
                        for k_ in range(int(R[b, s])):
                            ci = int(css[b, s]) + k_
                            gi, j = divmod(ci, K)
                            if gi != cur_gi[s]:
                                it = gp.tile([P, K * 8], i16, tag=f"i{s}",
                                             name="it")
                                nc.sync.dma_start(out=it[:],
                                                  in_=idx_ap[int(gioff[s]) + gi])
                                g_ = gp.tile([P, K * NF], f16, tag=f"g{s}",
                                             name="gt")
                                seg0 = s * cfg.SEGSZ
                                seg1 = min(seg0 + cfg.SEGSZ, src_rows)
                                nidx_eff = 128 if "gather" in skip else K * P
                                nc.gpsimd.dma_gather(
                                    out_ap=g_[:, :nidx_eff * NF // P].rearrange(
                                        "p (c f) -> p c f", f=NF),
                                    in_ap=src_ap[seg0:seg1, :],
                                    idxs_ap=it[:, :max(8, nidx_eff // 16)],
                                    num_idxs=nidx_eff,
                                    num_idxs_reg=nidx_eff,
                                    elem_size=NF,
                                )
                                dlt = gp.tile([P, K], f32, tag=f"d{s}",
                                              name="dlt")
                                nc.sync.dma_start(out=dlt[:],
                                                  in_=dl_ap[int(gioff[s]) + gi])
                                cur_gi[s], G[s], DL[s] = gi, g_, dlt
                            pt = ppool.tile([P, P], f16, tag="P", name="pt")
                            if "pts" not in skip or "mm" not in skip:
                                nc.vector.tensor_scalar(
                                    out=pt[:], in0=iota_t[:],
                                    scalar1=DL[s][:, j:j + 1], scalar2=None,
                                    op0=mybir.AluOpType.is_equal)
                            if "mm" not in skip:
                                nc.tensor.matmul(
                                    out=agg[:, :Freal], lhsT=pt[:],
                                    rhs=G[s][:, j * NF:j * NF + Freal],
                                    start=(t == 0), stop=(t == n_b - 1))
                            t += 1
                    if "epi" not in skip:
                        drain(b, agg)

            # ---------------- Layer 1 ----------------
            aggSall = const.tile([P, NB * NF], f16, name="aggSall")

            def drain1(b, agg):
                # single DVE op inline: scaled PSUM drain to SBUF staging
                nc.vector.tensor_scalar(
                    out=aggSall[:, b * NF:(b + 1) * NF], in0=agg[:],
                    scalar1=dinv_t[:, b:b + 1], scalar2=None,
                    op0=mybir.AluOpType.mult)

            def post1(b):
                dv = dinv_t[:, b:b + 1]
                aggS = aggSall[:, b * NF:(b + 1) * NF]
                aggT_ps = psB.tile([P, P], f16, space="PSUM", tag="tr",
                                   name="aggT_ps")
                nc.tensor.transpose(out=aggT_ps[:NF, :], in_=aggS,
                                    identity=ident[:])
                aggT = epi.tile([NF, P], f16, tag="aggT", name="aggT")
                nc.scalar.activation(out=aggT[:], in_=aggT_ps[:NF, :],
                                     func=mybir.ActivationFunctionType.Copy)
                t1_ps = psB.tile([P, H], f32, space="PSUM", tag="mm",
                                 name="t1_ps")
                nc.tensor.matmul(out=t1_ps[:], lhsT=aggT[:], rhs=w1_t[:],
                                 start=True, stop=True)
                tbl_in = epi.tile([P, H], f16, tag="tbl_in", name="tbl_in")
                if meta["has_b1"]:
                    t1b = epi.tile([P, H], f32, tag="t1b", name="t1b")
                    nc.vector.tensor_tensor(out=t1b[:], in0=t1_ps[:],
                                            in1=b1_t[:],
                                            op=mybir.AluOpType.add)
                    nc.vector.tensor_scalar(
                        out=tbl_in[:], in0=t1b[:], scalar1=dv, scalar2=0.0,
                        op0=mybir.AluOpType.mult, op1=mybir.AluOpType.max)
                else:
                    nc.vector.tensor_scalar(
                        out=tbl_in[:], in0=t1_ps[:], scalar1=dv, scalar2=0.0,
                        op0=mybir.AluOpType.mult, op1=mybir.AluOpType.max)
                tblT_ps = psB.tile([P, P], f16, space="PSUM", tag="tr",
                                   name="tblT_ps")
                nc.tensor.transpose(out=tblT_ps[:H, :], in_=tbl_in[:],
                                    identity=ident[:])
                tblT = epi.tile([H, P], f16, tag="tblT", name="tblT")
                nc.scalar.activation(out=tblT[:], in_=tblT_ps[:H, :],
                                     func=mybir.ActivationFunctionType.Copy)
                t2_ps = psB.tile([P, C], f32, space="PSUM", tag="mm",
                                 name="t2_ps")
                nc.tensor.matmul(out=t2_ps[:], lhsT=tblT[:], rhs=w2_t[:],
                                 start=True, stop=True)
                trow = epi.tile([P, C], f16, tag="trow", name="trow")
                nc.scalar.activation(out=trow[:], in_=t2_ps[:],
                                     func=mybir.ActivationFunctionType.Copy)
                nc.sync.dma_start(out=shard[b * P:(b + 1) * P, 0:C],
                                  in_=trow[:])

            # ---------------- Layer 2 epilogue ----------------
            def epi2(b, agg):
                dv = dinv_t[:, b:b + 1]
                res = epi.tile([P, C], f32, tag="res", name="res")
                if meta["has_b2"]:
                    r0 = epi.tile([P, C], f32, tag="r0", name="r0")
                    nc.vector.tensor_scalar(
                        out=r0[:], in0=agg[:, :C], scalar1=dv, scalar2=None,
                        op0=mybir.AluOpType.mult)
                    nc.vector.tensor_tensor(out=res[:], in0=r0[:], in1=b2_t[:],
                                            op=mybir.AluOpType.add)
                else:
                    nc.vector.tensor_scalar(
                        out=res[:], in0=agg[:, :C], scalar1=dv, scalar2=None,
                        op0=mybir.AluOpType.mult)
                nc.sync.dma_start(out=out[b * P:(b + 1) * P, :], in_=res[:])

            def whole_pipeline():
                layer(st1, idx1, dl1, x16, cfg.N, NF, drain1)
                if "epi" not in skip:
                    for b_ in range(NB):
                        post1(b_)
                if mock_cc:
                    for cc in range(cfg.NCORES):
                        nc.sync.dma_start(
                            out=table2p[cc * cfg.SHP:(cc + 1) * cfg.SHP, :],
                            in_=shard[:, :])
                else:
                    nc.gpsimd.collective_compute(
                        "AllGather",
                        mybir.AluOpType.bypass,
                        replica_groups=[list(range(cfg.NCORES))],
                        ins=[shard.opt()],
                        outs=[table2p.opt()],
                    )
                layer(st2, idx2, dl2, table2p[:, :], cfg.TROW, C, epi2)

            if repeat == 1:
                whole_pipeline()
            else:
                with tc.For_i(0, repeat, 1):
                    whole_pipeline()
    return nc


def run_device(cfg, meta, ins, trace=False, repeat=1, mock_cc=False):
    from concourse import bass_utils
    nc = bacc.Bacc("TRN2", target_bir_lowering=False, debug=False,
                   num_devices=cfg.NCORES)
    build_kernel(cfg, meta, nc, repeat=repeat, mock_cc=mock_cc)
    nc.compile()
    in_maps = []
    for c in range(cfg.NCORES):
        in_maps.append(dict(
            x16=ins["x16"], idx1=ins["idx1"][c], dl1=ins["dl1"][c],
            idx2=ins["idx2"][c], dl2=ins["dl2"][c],
            dinv_cols=ins["dinv_cols"][c], w1=ins["w1"], w2=ins["w2"],
            b1=ins["b1"], b2=ins["b2"], iota=ins["iota"], ident=ins["ident"],
        ))
    res = bass_utils.run_bass_kernel_spmd(
        nc, in_maps, core_ids=list(range(cfg.NCORES)), trace=trace)
    return res


def assemble_output(cfg, meta, results):
    parts = [results[c]["out"][:cfg.SH] for c in range(cfg.NCORES)]
    return np.concatenate(parts, 0).astype(np.float32)


def kernel(x, edge_index, W1, b1, W2, b2):
    cfg = CFG()
    meta, ins = preprocess(cfg, x, edge_index, W1, b1, W2, b2)
    res = run_device(cfg, meta, ins, trace=False)
    return assemble_output(cfg, meta, res.results)
